# revision 1
# baseline (speedup 1.0000x reference)
"""Trainium2 Bass kernel for nn_ConstrainNet (block-banded dynamics residual).

Reference computation (n_state=64, n_input=32, n_all=96, T=128):
    V = net_input.reshape(T, 96)
    out block 0      = V[0, :64] - x0
    out block t+1    = [A B] @ V[t] - V[t+1, :64]        (t = 0..T-2)
    output = concat of the 128 blocks -> (8192,) f32

Sharding: time axis split across 8 NeuronCores; core k computes output
blocks t in [16k, 16k+16). Inputs arrive FULL on host, so the one-step
"halo" is just an overlapping host-side slice — no collectives needed.

The whole per-core computation is ONE augmented matmul with contraction
K = 96 + 1 + 16 = 113:
    out[j, s] = sum_a lhsT[a, j] * rhs[a, s]
      rows  0..95 : lhsT = Vm^T, rhs = [A B]^T          -> AB @ Vm[j]
      row     96  : identity-block fixup (core 0 only):
                    lhsT[96, 0] = 1, rhs[96, :] = V[0, :64]
      rows 97..112: lhsT[97+j', j] = -delta(j', j), rhs[97+j] = S[j]
                    -> subtracts S[j] (= V[t+1, :64]; x0 for block 0)
All augmentation entries are constants or pure host-side slices — no
host arithmetic.

Device-side layout tuning (HWDGE moves one packet per SBUF partition,
~19 GB/s per queue, so fewer/fatter packets win): the host packs TWO
K-rows per partition — DRAM tensor w[57, 160] with
    w[p,   0: 64] = rhs row p        w[p,  64: 80] = lhsT row p
    w[p,  80:144] = rhs row 57+p     w[p, 144:160] = lhsT row 57+p
(row 56 of the second group is zero padding). One 36.5KB DMA of 57
640B-packets, then two PSUM-accumulating matmuls (K=57 each; the zero
pad row contributes nothing), a DVE copy PSUM->SBUF, and the output
store, whose descriptor generation overlaps the copy (see the comment
at the store) and whose completion is not waited on (the runtime
quiesces DMA before output readback).

Raw Bass (no TileContext): this walrus build rejects instructions that
carry more than one sync wait, and Tile's end-of-context drain
aggregates one wait per live semaphore. The manual chain below carries
at most one wait per instruction. Measured: ~12.06us NEFF exec time,
~0.3us above an empty DMA-in/DMA-out kernel on the same toolchain.
"""

import numpy as np

N_STATE = 64
N_INPUT = 32
N_ALL = N_STATE + N_INPUT  # 96
T_FULL = 128
N_CORES = 8
TB = T_FULL // N_CORES  # 16 output blocks per core
K = N_ALL + 1 + TB  # 113 contraction rows
W_COLS = N_STATE + TB  # 80: [rhs | lhsT] packed along the free dim
KP = 57  # packed partitions: K-rows p and 57+p share partition p

_PROGRAM_CACHE = {}


def _build_program():
    import concourse.bass as bass
    import concourse.mybir as mybir

    f32 = mybir.dt.float32
    nc = bass.Bass("TRN2", debug=False)

    w = nc.dram_tensor("w", [KP, 2 * W_COLS], f32, kind="ExternalInput")
    out_d = nc.dram_tensor("out", [TB, N_STATE], f32, kind="ExternalOutput")

    # Instructions are emitted straight into the main block (no nc.Block()):
    # the per-engine branch into a Block basic block costs ~400ns on the
    # critical path. Each engine executes only its own instructions, in
    # program order, so the semaphore chain below is unchanged.
    with (
        nc.sbuf_tensor([KP, 2 * W_COLS], f32) as w_t,
        nc.psum_tensor([TB, N_STATE], f32) as acc,
        nc.sbuf_tensor([TB, N_STATE], f32) as o_t,
        nc.semaphore("dma_a") as dma_a,
        nc.semaphore("mm") as mm,
        nc.semaphore("cp") as cp,
        nc.semaphore("dma_out") as dma_out,
    ):
        nc.sync.dma_start(out=w_t[:], in_=w[:]).then_inc(dma_a, 16)
        nc.tensor.wait_ge(dma_a, 16)
        nc.tensor.matmul(
            acc[:],
            w_t[0:KP, N_STATE:W_COLS],
            w_t[0:KP, 0:N_STATE],
            start=True,
            stop=False,
        )
        # second group's row 56 is zero padding -> contributes nothing
        nc.tensor.matmul(
            acc[:],
            w_t[0:KP, W_COLS + N_STATE : 2 * W_COLS],
            w_t[0:KP, W_COLS : W_COLS + N_STATE],
            start=False,
            stop=True,
        ).then_inc(mm, 1)
        nc.vector.wait_ge(mm, 1)
        nc.vector.tensor_copy(o_t[:], acc[:]).then_inc(cp, 1)
        # The store's descriptor generation is gated on mm, not cp: DMA
        # descriptors encode addresses only, and the HWDGE ring launch
        # (~1.4us from desc-gen start to first data read) dwarfs the DVE
        # copy (~350ns after mm), so the transfer reads o_t well after the
        # copy lands (measured margin ~1.0us). This overlaps desc-gen with
        # the copy instead of serializing behind it.
        nc.sync.wait_ge(mm, 1)
        nc.sync.dma_start(out=out_d[:], in_=o_t[:]).then_inc(dma_out, 16)

    return nc


def _get_program():
    if "nc" not in _PROGRAM_CACHE:
        _PROGRAM_CACHE["nc"] = _build_program()
    return _PROGRAM_CACHE["nc"]


def _make_in_maps(A, B, x0, net_input):
    A = np.ascontiguousarray(A, dtype=np.float32)
    B = np.ascontiguousarray(B, dtype=np.float32)
    x0 = np.ascontiguousarray(x0, dtype=np.float32)
    V = np.ascontiguousarray(net_input, dtype=np.float32).reshape(T_FULL, N_ALL)

    ab_t = np.concatenate([A, B], axis=1).T  # (96, 64)

    in_maps = []
    for k in range(N_CORES):
        w = np.zeros((K, W_COLS), dtype=np.float32)
        rhs = w[:, :N_STATE]
        lhsT = w[:, N_STATE:]
        rhs[:N_ALL] = ab_t
        # rows 97..112: -I in lhsT, S rows in rhs
        lhsT[N_ALL + 1 :] = -np.eye(TB, dtype=np.float32)
        t0 = k * TB
        if k == 0:
            rhs[N_ALL] = V[0, :N_STATE]  # identity-block fixup
            lhsT[N_ALL, 0] = 1.0
            lhsT[:N_ALL, 1:] = V[0 : TB - 1].T
            rhs[N_ALL + 1] = x0
            rhs[N_ALL + 2 :] = V[1:TB, :N_STATE]
        else:
            lhsT[:N_ALL] = V[t0 - 1 : t0 + TB - 1].T
            rhs[N_ALL + 1 :] = V[t0 : t0 + TB, :N_STATE]
        # pack two K-rows per partition: [row p | row 57+p]
        w2 = np.zeros((KP, 2 * W_COLS), dtype=np.float32)
        w2[:, :W_COLS] = w[0:KP]
        w2[0 : K - KP, W_COLS:] = w[KP:K]
        in_maps.append({"w": w2})
    return in_maps


def kernel(A, B, x0, net_input, T):
    assert int(T) == T_FULL, f"kernel hardcoded for T={T_FULL}, got {T}"
    from concourse.bass_utils import run_bass_kernel_spmd

    nc = _get_program()
    in_maps = _make_in_maps(A, B, x0, net_input)
    res = run_bass_kernel_spmd(nc, in_maps, core_ids=list(range(N_CORES)))
    out = np.concatenate([np.asarray(r["out"]).reshape(-1) for r in res.results])
    return out.astype(np.float32)



# revision 15
# speedup vs baseline: 1.1136x; 1.1136x over previous
"""Trainium2 Bass kernel for nn_ConstrainNet (block-banded dynamics residual).

Reference computation (n_state=64, n_input=32, n_all=96, T=128):
    V = net_input.reshape(T, 96)
    out block 0      = V[0, :64] - x0
    out block t+1    = [A B] @ V[t] - V[t+1, :64]        (t = 0..T-2)
    output = concat of the 128 blocks -> (8192,) f32

Sharding: time axis split across 8 NeuronCores; core k computes output
blocks t in [16k, 16k+16). Inputs arrive FULL on host, so the one-step
"halo" is just an overlapping host-side slice — no collectives needed.

Per-core computation, layout "aug" (single augmented matmul, K = 96+1+16):
    out[j, s] = sum_a lhsT[a, j] * rhs[a, s]
      rows  0..95 : lhsT = Vm^T, rhs = [A B]^T          -> AB @ Vm[j]
      row     96  : identity-block fixup (core 0 only)
      rows 97..112: -I in lhsT, S rows in rhs -> subtracts S[j]
Layout "sub" drops the 16 "-I" rows (K = 97) and subtracts S on the DVE
(tensor_sub of the PSUM accumulator and an SBUF tile) instead.

Perf notes (see memory: trn2-exec-time-model). Measured vs the 12.2us
first-generation kernel: ~11.1us mean, ~10.75us best (process-level
clock/NRT variance dominates the spread):
  * bf16 operands (tolerance 2e-2, lands 2.3e-3): halves DMA payload and
    runs matmuls single-pass instead of fp32 LOW/HIGH dual-pass
    (~0.3us tensor-engine saving).
  * K-rows packed two per SBUF partition; the load is split 48/9 across
    the two HWDGE engines (Sync gets most rows — Activation desc-gen has
    a fixed ~1.4us cost regardless of descriptor count, but its transfer
    overlaps Sync's) incrementing ONE shared semaphore (threshold 32).
  * Store desc-gen gated on the INPUT semaphore, not the matmul:
    descriptors encode addresses only, and the HWDGE pipe (desc-gen
    ~0.7us + DGE start delay ~0.65us) reaches its first data read well
    after the result copy lands (~0.5us margin, stable because copy and
    desc-gen are gated on the same semaphore). The store desc-gen thus
    fully overlaps the matmul+copy (~0.7us saving).
  * Single user semaphore (matmul and both DMAs all increment dma_a) and
    Bass(enable_partition_id=False, monotonic_sem_count=0): the
    end-of-NEFF semaphore-reset chain is inside the measured window, so
    fewer live semaphores ends the profile window earlier.

Raw Bass (no TileContext): this walrus build rejects instructions that
carry more than one sync wait, and walrus codegen requires every dynamic
DMA to carry a completion semaphore.
"""

import numpy as np

N_STATE = 64
N_INPUT = 32
N_ALL = N_STATE + N_INPUT  # 96
T_FULL = 128
N_CORES = 8
TB = T_FULL // N_CORES  # 16 output blocks per core
W_COLS = N_STATE + TB  # 80: [rhs | lhsT] packed along the free dim

# layout "aug": K=113 rows packed 2/partition -> 57 partitions x 160 cols
K_AUG = N_ALL + 1 + TB
KP_AUG = 57
WC_AUG = 2 * W_COLS  # 160
# layout "sub": K=97 rows packed 2/partition -> 49 partitions, plus the
# 16x64 subtrahend block at cols 160:224 (rows 0:16)
K_SUB = N_ALL + 1
KP_SUB = 49
WC_SUB = 2 * W_COLS + N_STATE  # 224

_PROGRAM_CACHE = {}

# (ka, out_engine, nsems, layout, minflags):
#   ka        = partition rows loaded by the Sync HWDGE queue (rest by
#               Scalar; ka == KP means no split)
#   out_engine= engine issuing the store desc-gen ("sync" | "scalar")
#   nsems     = 1 (single shared semaphore) | 3 (dma_a / mm / dma_out)
#   layout    = "aug" | "sub"
#   minflags  = True -> Bass(enable_partition_id=False, monotonic_sem_count=0)
DEFAULT_CFG = (48, "sync", 1, "aug", True)


def _kp(layout):
    return KP_AUG if layout == "aug" else KP_SUB


def _wc(layout):
    return WC_AUG if layout == "aug" else WC_SUB


def _build_program(cfg=DEFAULT_CFG):
    from contextlib import ExitStack

    import concourse.bass as bass
    import concourse.mybir as mybir

    ka, out_engine, nsems, layout, minflags = cfg
    kp, wc = _kp(layout), _wc(layout)
    f32 = mybir.dt.float32
    bf16 = mybir.dt.bfloat16
    kwargs = (
        dict(enable_partition_id=False, monotonic_sem_count=0) if minflags else {}
    )
    nc = bass.Bass("TRN2", debug=False, **kwargs)

    wa = nc.dram_tensor("wa", [ka, wc], bf16, kind="ExternalInput")
    wb = (
        nc.dram_tensor("wb", [kp - ka, wc], bf16, kind="ExternalInput")
        if ka < kp
        else None
    )
    out_d = nc.dram_tensor("out", [TB, N_STATE], f32, kind="ExternalOutput")

    # Instructions are emitted straight into the main block (no nc.Block()):
    # the per-engine branch into a Block basic block costs ~400ns on the
    # critical path.
    ctx = ExitStack()
    w_t = ctx.enter_context(nc.sbuf_tensor([kp, wc], bf16))
    acc = ctx.enter_context(nc.psum_tensor([TB, N_STATE], f32))
    o_t = ctx.enter_context(nc.sbuf_tensor([TB, N_STATE], f32))
    dma_a = ctx.enter_context(nc.semaphore("dma_a"))
    if nsems == 1:
        mm = dma_out = dma_a
    else:
        mm = ctx.enter_context(nc.semaphore("mm"))
        dma_out = ctx.enter_context(nc.semaphore("dma_out"))

    nc.sync.dma_start(out=w_t[0:ka, :], in_=wa[:]).then_inc(dma_a, 16)
    if wb is not None:
        nc.scalar.dma_start(out=w_t[ka:kp, :], in_=wb[:]).then_inc(dma_a, 16)
        thr = 32
    else:
        thr = 16
    nc.tensor.wait_ge(dma_a, thr)
    nc.tensor.matmul(
        acc[:],
        w_t[0:kp, N_STATE:W_COLS],
        w_t[0:kp, 0:N_STATE],
        start=True,
        stop=False,
    )
    # second group's last row is zero padding -> contributes nothing
    nc.tensor.matmul(
        acc[:],
        w_t[0:kp, W_COLS + N_STATE : 2 * W_COLS],
        w_t[0:kp, W_COLS : W_COLS + N_STATE],
        start=False,
        stop=True,
    ).then_inc(mm, 1)
    # result producer fires on matmul-done (threshold thr+1 when shared)
    nc.vector.wait_ge(mm, thr + 1 if nsems == 1 else 1)
    if layout == "aug":
        nc.vector.tensor_copy(o_t[:], acc[:])
    else:
        nc.vector.tensor_sub(o_t[:], acc[:], w_t[0:TB, 2 * W_COLS : WC_SUB])
    eng = {"scalar": nc.scalar, "gpsimd": nc.gpsimd}.get(out_engine, nc.sync)
    # Store desc-gen gated on input-data ready (not matmul): descriptors
    # encode addresses only, and the HWDGE pipe (desc-gen + DGE start
    # delay) reaches its first data read well after o_t lands.
    eng.wait_ge(dma_a, thr)
    eng.dma_start(out=out_d[:], in_=o_t[:]).then_inc(dma_out, 16)

    ctx.close()
    return nc


def _get_program(cfg=DEFAULT_CFG):
    if cfg not in _PROGRAM_CACHE:
        _PROGRAM_CACHE[cfg] = _build_program(cfg)
    return _PROGRAM_CACHE[cfg]


def _make_in_maps(A, B, x0, net_input, ka=None, layout=None):
    import ml_dtypes

    if ka is None:
        ka = DEFAULT_CFG[0]
    if layout is None:
        layout = DEFAULT_CFG[3]
    kp, wc = _kp(layout), _wc(layout)
    k_rows = K_AUG if layout == "aug" else K_SUB

    A = np.ascontiguousarray(A, dtype=np.float32)
    B = np.ascontiguousarray(B, dtype=np.float32)
    x0 = np.ascontiguousarray(x0, dtype=np.float32)
    V = np.ascontiguousarray(net_input, dtype=np.float32).reshape(T_FULL, N_ALL)

    ab_t = np.concatenate([A, B], axis=1).T  # (96, 64)

    in_maps = []
    for k in range(N_CORES):
        w = np.zeros((k_rows, W_COLS), dtype=np.float32)
        rhs = w[:, :N_STATE]
        lhsT = w[:, N_STATE:]
        rhs[:N_ALL] = ab_t
        t0 = k * TB
        # subtrahend rows: S[0] is x0 on core 0, else V[t0]; S[j>0]=V[t0+j]
        s_rows = np.empty((TB, N_STATE), dtype=np.float32)
        if k == 0:
            rhs[N_ALL] = V[0, :N_STATE]  # identity-block fixup
            lhsT[N_ALL, 0] = 1.0
            lhsT[:N_ALL, 1:] = V[0 : TB - 1].T
            s_rows[0] = x0
            s_rows[1:] = V[1:TB, :N_STATE]
        else:
            lhsT[:N_ALL] = V[t0 - 1 : t0 + TB - 1].T
            s_rows[:] = V[t0 : t0 + TB, :N_STATE]
        if layout == "aug":
            # rows 97..112: -I in lhsT, S rows in rhs
            lhsT[N_ALL + 1 :] = -np.eye(TB, dtype=np.float32)
            rhs[N_ALL + 1 :] = s_rows
        # pack two K-rows per partition: [row p | row kp+p]
        w2 = np.zeros((kp, wc), dtype=np.float32)
        w2[:, :W_COLS] = w[0:kp]
        w2[0 : k_rows - kp, W_COLS : 2 * W_COLS] = w[kp:k_rows]
        if layout == "sub":
            w2[0:TB, 2 * W_COLS :] = s_rows
        w2 = w2.astype(ml_dtypes.bfloat16)
        m = {"wa": np.ascontiguousarray(w2[0:ka])}
        if ka < kp:
            m["wb"] = np.ascontiguousarray(w2[ka:kp])
        in_maps.append(m)
    return in_maps


def kernel(A, B, x0, net_input, T):
    assert int(T) == T_FULL, f"kernel hardcoded for T={T_FULL}, got {T}"
    from concourse.bass_utils import run_bass_kernel_spmd

    nc = _get_program()
    in_maps = _make_in_maps(A, B, x0, net_input)
    res = run_bass_kernel_spmd(nc, in_maps, core_ids=list(range(N_CORES)))
    out = np.concatenate([np.asarray(r["out"]).reshape(-1) for r in res.results])
    return out.astype(np.float32)


# revision 21
# speedup vs baseline: 1.1178x; 1.0038x over previous
"""Trainium2 Bass kernel for nn_ConstrainNet (block-banded dynamics residual).

Reference computation (n_state=64, n_input=32, n_all=96, T=128):
    V = net_input.reshape(T, 96)
    out block 0      = V[0, :64] - x0
    out block t+1    = [A B] @ V[t] - V[t+1, :64]        (t = 0..T-2)
    output = concat of the 128 blocks -> (8192,) f32

Sharding: time axis split across 8 NeuronCores; core k computes output
blocks t in [16k, 16k+16). Inputs arrive FULL on host, so the one-step
"halo" is just an overlapping host-side slice — no collectives needed.

Per-core computation, layout "aug" (single augmented matmul, K = 96+1+16):
    out[j, s] = sum_a lhsT[a, j] * rhs[a, s]
      rows  0..95 : lhsT = Vm^T, rhs = [A B]^T          -> AB @ Vm[j]
      row     96  : identity-block fixup (core 0 only)
      rows 97..112: -I in lhsT, S rows in rhs -> subtracts S[j]
Layout "sub" drops the 16 "-I" rows (K = 97) and subtracts S on the DVE
(tensor_sub of the PSUM accumulator and an SBUF tile) instead.

Perf notes (see memory: trn2-exec-time-model). Measured vs the 12.2us
first-generation kernel: ~11.1us mean, ~10.75us best (process-level
clock/NRT variance dominates the spread):
  * bf16 operands (tolerance 2e-2, lands 2.3e-3): halves DMA payload and
    runs matmuls single-pass instead of fp32 LOW/HIGH dual-pass
    (~0.3us tensor-engine saving).
  * K-rows packed two per SBUF partition; the load is split 48/9 across
    the two HWDGE engines (Sync gets most rows — Activation desc-gen has
    a fixed ~1.4us cost regardless of descriptor count, but its transfer
    overlaps Sync's) incrementing ONE shared semaphore (threshold 32).
  * Store desc-gen gated on the INPUT semaphore, not the matmul:
    descriptors encode addresses only, and the HWDGE pipe (desc-gen
    ~0.7us + DGE start delay ~0.65us) reaches its first data read well
    after the result copy lands (~0.5us margin, stable because copy and
    desc-gen are gated on the same semaphore). The store desc-gen thus
    fully overlaps the matmul+copy (~0.7us saving).
  * Single user semaphore (matmul and both DMAs all increment dma_a) and
    Bass(enable_partition_id=False, monotonic_sem_count=0): the
    end-of-NEFF semaphore-reset chain is inside the measured window, so
    fewer live semaphores ends the profile window earlier.
  * single_packet=True on both Sync DMAs: ~0.2us each in-process,
    margin-neutral (measured first-output-read vs copy-land unchanged).
  * REJECTED: gating the store desc-gen on dma_a>=24 (partial input
    completion) measures ~0.43us faster, but in slow-mode runs the last
    8 input packets straggle ~1.5us behind (HBM/NRT contention), which
    would blow the ~0.4us race margin and corrupt the output. Do not
    lower the store gate below full input completion.

Raw Bass (no TileContext): this walrus build rejects instructions that
carry more than one sync wait, and walrus codegen requires every dynamic
DMA to carry a completion semaphore.
"""

import numpy as np

N_STATE = 64
N_INPUT = 32
N_ALL = N_STATE + N_INPUT  # 96
T_FULL = 128
N_CORES = 8
TB = T_FULL // N_CORES  # 16 output blocks per core
W_COLS = N_STATE + TB  # 80: [rhs | lhsT] packed along the free dim

# layout "aug": K=113 rows packed 2/partition -> 57 partitions x 160 cols
K_AUG = N_ALL + 1 + TB
KP_AUG = 57
WC_AUG = 2 * W_COLS  # 160
# layout "sub": K=97 rows packed 2/partition -> 49 partitions, plus the
# 16x64 subtrahend block at cols 160:224 (rows 0:16)
K_SUB = N_ALL + 1
KP_SUB = 49
WC_SUB = 2 * W_COLS + N_STATE  # 224

_PROGRAM_CACHE = {}

# (ka, out_engine, nsems, layout, minflags, out_thr, sp_out, sp_in):
#   ka        = partition rows loaded by the Sync HWDGE queue (rest by
#               Scalar; ka == KP means no split)
#   out_engine= engine issuing the store desc-gen ("sync" | "scalar")
#   nsems     = 1 (single shared semaphore) | 3 (dma_a / mm / dma_out)
#   layout    = "aug" | "sub"
#   minflags  = True -> Bass(enable_partition_id=False, monotonic_sem_count=0)
#   out_thr   = store desc-gen gate threshold on dma_a (None = input done)
#   sp_out/sp_in = single_packet on the store / Sync-load DMA
DEFAULT_CFG = (48, "sync", 1, "aug", True, None, True, True)


def _kp(layout):
    return KP_AUG if layout == "aug" else KP_SUB


def _wc(layout):
    return WC_AUG if layout == "aug" else WC_SUB


def _build_program(cfg=DEFAULT_CFG):
    from contextlib import ExitStack

    import concourse.bass as bass
    import concourse.mybir as mybir

    ka, out_engine, nsems, layout, minflags, out_thr, sp_out, sp_in = cfg
    kp, wc = _kp(layout), _wc(layout)
    f32 = mybir.dt.float32
    bf16 = mybir.dt.bfloat16
    kwargs = (
        dict(enable_partition_id=False, monotonic_sem_count=0) if minflags else {}
    )
    nc = bass.Bass("TRN2", debug=False, **kwargs)

    wa = nc.dram_tensor("wa", [ka, wc], bf16, kind="ExternalInput")
    wb = (
        nc.dram_tensor("wb", [kp - ka, wc], bf16, kind="ExternalInput")
        if ka < kp
        else None
    )
    out_d = nc.dram_tensor("out", [TB, N_STATE], f32, kind="ExternalOutput")

    # Instructions are emitted straight into the main block (no nc.Block()):
    # the per-engine branch into a Block basic block costs ~400ns on the
    # critical path.
    ctx = ExitStack()
    w_t = ctx.enter_context(nc.sbuf_tensor([kp, wc], bf16))
    acc = ctx.enter_context(nc.psum_tensor([TB, N_STATE], f32))
    o_t = ctx.enter_context(nc.sbuf_tensor([TB, N_STATE], f32))
    dma_a = ctx.enter_context(nc.semaphore("dma_a"))
    if nsems == 1:
        mm = dma_out = dma_a
    else:
        mm = ctx.enter_context(nc.semaphore("mm"))
        dma_out = ctx.enter_context(nc.semaphore("dma_out"))

    nc.sync.dma_start(out=w_t[0:ka, :], in_=wa[:], single_packet=sp_in).then_inc(
        dma_a, 16
    )
    if wb is not None:
        nc.scalar.dma_start(out=w_t[ka:kp, :], in_=wb[:]).then_inc(dma_a, 16)
        thr = 32
    else:
        thr = 16
    nc.tensor.wait_ge(dma_a, thr)
    nc.tensor.matmul(
        acc[:],
        w_t[0:kp, N_STATE:W_COLS],
        w_t[0:kp, 0:N_STATE],
        start=True,
        stop=False,
    )
    # second group's last row is zero padding -> contributes nothing
    nc.tensor.matmul(
        acc[:],
        w_t[0:kp, W_COLS + N_STATE : 2 * W_COLS],
        w_t[0:kp, W_COLS : W_COLS + N_STATE],
        start=False,
        stop=True,
    ).then_inc(mm, 1)
    # result producer fires on matmul-done (threshold thr+1 when shared)
    nc.vector.wait_ge(mm, thr + 1 if nsems == 1 else 1)
    if layout == "aug":
        nc.vector.tensor_copy(o_t[:], acc[:])
    else:
        nc.vector.tensor_sub(o_t[:], acc[:], w_t[0:TB, 2 * W_COLS : WC_SUB])
    eng = {"scalar": nc.scalar, "gpsimd": nc.gpsimd}.get(out_engine, nc.sync)
    # Store desc-gen gated on input-data ready (not matmul): descriptors
    # encode addresses only, and the HWDGE pipe (desc-gen + DGE start
    # delay) reaches its first data read well after o_t lands.
    eng.wait_ge(dma_a, thr if out_thr is None else out_thr)
    eng.dma_start(out=out_d[:], in_=o_t[:], single_packet=sp_out).then_inc(dma_out, 16)

    ctx.close()
    return nc


def _get_program(cfg=DEFAULT_CFG):
    if cfg not in _PROGRAM_CACHE:
        _PROGRAM_CACHE[cfg] = _build_program(cfg)
    return _PROGRAM_CACHE[cfg]


def _make_in_maps(A, B, x0, net_input, ka=None, layout=None):
    import ml_dtypes

    if ka is None:
        ka = DEFAULT_CFG[0]
    if layout is None:
        layout = DEFAULT_CFG[3]
    kp, wc = _kp(layout), _wc(layout)
    k_rows = K_AUG if layout == "aug" else K_SUB

    A = np.ascontiguousarray(A, dtype=np.float32)
    B = np.ascontiguousarray(B, dtype=np.float32)
    x0 = np.ascontiguousarray(x0, dtype=np.float32)
    V = np.ascontiguousarray(net_input, dtype=np.float32).reshape(T_FULL, N_ALL)

    ab_t = np.concatenate([A, B], axis=1).T  # (96, 64)

    in_maps = []
    for k in range(N_CORES):
        w = np.zeros((k_rows, W_COLS), dtype=np.float32)
        rhs = w[:, :N_STATE]
        lhsT = w[:, N_STATE:]
        rhs[:N_ALL] = ab_t
        t0 = k * TB
        # subtrahend rows: S[0] is x0 on core 0, else V[t0]; S[j>0]=V[t0+j]
        s_rows = np.empty((TB, N_STATE), dtype=np.float32)
        if k == 0:
            rhs[N_ALL] = V[0, :N_STATE]  # identity-block fixup
            lhsT[N_ALL, 0] = 1.0
            lhsT[:N_ALL, 1:] = V[0 : TB - 1].T
            s_rows[0] = x0
            s_rows[1:] = V[1:TB, :N_STATE]
        else:
            lhsT[:N_ALL] = V[t0 - 1 : t0 + TB - 1].T
            s_rows[:] = V[t0 : t0 + TB, :N_STATE]
        if layout == "aug":
            # rows 97..112: -I in lhsT, S rows in rhs
            lhsT[N_ALL + 1 :] = -np.eye(TB, dtype=np.float32)
            rhs[N_ALL + 1 :] = s_rows
        # pack two K-rows per partition: [row p | row kp+p]
        w2 = np.zeros((kp, wc), dtype=np.float32)
        w2[:, :W_COLS] = w[0:kp]
        w2[0 : k_rows - kp, W_COLS : 2 * W_COLS] = w[kp:k_rows]
        if layout == "sub":
            w2[0:TB, 2 * W_COLS :] = s_rows
        w2 = w2.astype(ml_dtypes.bfloat16)
        m = {"wa": np.ascontiguousarray(w2[0:ka])}
        if ka < kp:
            m["wb"] = np.ascontiguousarray(w2[ka:kp])
        in_maps.append(m)
    return in_maps


def kernel(A, B, x0, net_input, T):
    assert int(T) == T_FULL, f"kernel hardcoded for T={T_FULL}, got {T}"
    from concourse.bass_utils import run_bass_kernel_spmd

    nc = _get_program()
    in_maps = _make_in_maps(A, B, x0, net_input)
    res = run_bass_kernel_spmd(nc, in_maps, core_ids=list(range(N_CORES)))
    out = np.concatenate([np.asarray(r["out"]).reshape(-1) for r in res.results])
    return out.astype(np.float32)
